# revision 21
# baseline (speedup 1.0000x reference)
"""Trainium2 Bass kernel for nn_CSG2A_net (gnn_message_passing).

Math (algebraically identical to the reference; the [B,G,G] score tensor is
never materialized):
  CCE:  h = relu(node_feat @ W1); w = adj*exp(-dist)
        g[b,m] = sum_n mask[b,n] * w[b,n,m]
        pooled[d,b] = (sum_m g[b,m] h[b,m,d]) / clip(sum_n mask[b,n], 1)
        comp = pooled @ W2 + dose @ w_dose + time @ w_time
  score.sum(-1)[b,g] = q[b,g,:] . u[b,:] / sqrt(H),  u = b_gex@w_gex + comp@w_comp
  pred = b_gex * (ssum + ppi_adj.sum(-1));  out = relu(LN(pred)) @ W_ff

Sharding: data-parallel over batch across 8 cores (8 samples each), weights
replicated.  On-chip layout is gene-major ([G-tile partitions x batch free]).

The cost structure on TRN2 is dominated by serialized HBM DMA (~360 B/ns all
queues combined), so the kernel minimizes DMA bytes and DMA count:
  - weights are down-cast host-side: w_gex/w_comp/W_ff/CCE weights to bf16,
    ppi_adj to fp8(e3m4) (it only feeds row-sums; quantization error on a
    978-element sum of U[0,1) values is ~0.04% of the sum)
  - ppi is staged TRANSPOSED so its row sums contract over the partition dim:
    64 rank-reduced PE matmuls against a ones vector instead of ~8us of
    DVE/ACT free-dim reductions
  - all small inputs ride in 4 packed images of the SBUF destination tiles
    (one DMA each); outputs pack into one [128,128] f32 tile (one DMA)
  - FFN runs transposed (out^T = W_ff^T x^T per gene tile) so each matmul
    moves only 8 rows; W_ff streams in 3 chunks overlapped with compute
"""

import numpy as np
import ml_dtypes

import concourse.bass as bass
import concourse.mybir as mybir
import concourse.tile as tile
from concourse.bass_utils import run_bass_kernel_spmd
from concourse.masks import make_identity

F32 = mybir.dt.float32
BF16 = mybir.dt.bfloat16
F8 = mybir.dt.float8e3
AF = mybir.ActivationFunctionType

NP_BF16 = ml_dtypes.bfloat16
NP_F8 = ml_dtypes.float8_e3m4

G, H, NA, FEAT, CH = 978, 128, 50, 34, 64
B, NCORES = 64, 8
BL = B // NCORES  # per-core batch
LN_EPS = 1e-5
# gene-dim tiles: 7 x 128 + 82
GTS = [(i * 128, 128) for i in range(7)] + [(896, 82)]
NGT = len(GTS)

# pack50 column layout: nfT | adjT | distT | W1 | maskT
P50_NF, P50_ADJ, P50_DIST, P50_W1, P50_MASK = 0, 400, 800, 1200, 1264
P50_W = 1272
# pack1 column layout: w_dose | w_time | doseT | timeT
P1_WD, P1_WT, P1_DO, P1_TI = 0, G, 2 * G, 2 * G + BL
P1_W = 2 * G + 2 * BL
# pack128 column layout: w_gex tiles | w_comp tiles | b_gex^T (bf16)
P128_WG, P128_WC, P128_BGT = 0, NGT * H, 2 * NGT * H
P128_W = 2 * NGT * H + NGT * BL
# packf (f32) column layout: b_gex^T tiles | ln_gamma | ln_beta
PF_BGT, PF_GAM, PF_BET = 0, NGT * BL, NGT * BL + NGT
PF_W = NGT * BL + 2 * NGT

_DMA_ZERO_WAIT = ("InstDMACopy", "InstDMATransposeAnt", "InstTriggeredCopy")


def _split_excess_waits(nc):
    """walrus in this container accepts at most 1 inline sync-wait per
    instruction (0 for DMA).  Move excess waits onto same-engine nops
    inserted immediately before the overloaded instruction."""

    def make_nop(engine):
        bi = nc.engines[engine].nop(nofuse=True)
        ins = bi.ins
        lst = nc.cur_bb.bb.instructions
        assert lst[-1] is ins
        lst.pop()
        return ins

    for bb in nc.main_func.blocks:
        lst = bb.instructions
        i = 0
        while i < len(lst):
            ins = lst[i]
            si = getattr(ins, "sync_info", None)
            waits = list(si.on_wait) if (si and si.on_wait) else []
            limit = 0 if type(ins).__name__ in _DMA_ZERO_WAIT else 1
            if len(waits) > limit:
                keep = waits[len(waits) - limit:] if limit else []
                excess = waits[: len(waits) - limit]
                si.on_wait = keep
                pos = i
                for w in excess:
                    nop = make_nop(ins.engine)
                    nop.sync_info = mybir.SyncInfo(on_wait=[w], on_update=[])
                    lst.insert(pos, nop)
                    pos += 1
                    i += 1
            i += 1


def build_nc():
    nc = bass.Bass()

    # ---- kernel I/O (per-core; all host-packed) ----
    pack50 = nc.dram_tensor("pack50", [NA, P50_W], BF16, kind="ExternalInput")
    pack1 = nc.dram_tensor("pack1", [1, P1_W], BF16, kind="ExternalInput")
    pack64 = nc.dram_tensor("pack64", [CH, G], BF16, kind="ExternalInput")
    pack128 = nc.dram_tensor("pack128", [128, P128_W], BF16, kind="ExternalInput")
    packf = nc.dram_tensor("packf", [128, PF_W], F32, kind="ExternalInput")
    ppiT8 = nc.dram_tensor("ppiT8", [G, G], F8, kind="ExternalInput")
    wffb = nc.dram_tensor("wffb", [G, G], BF16, kind="ExternalInput")
    # outs: cols [0:64] = pred^T tiles (t*BL+b), [64:128] = comp^T tiles
    outs = nc.dram_tensor("outs", [128, 128], F32, kind="ExternalOutput")

    inv_sqrt_h = 1.0 / float(np.sqrt(H))

    with tile.TileContext(nc) as tc:
        with (
            tc.tile_pool(name="const", bufs=1) as const,
            tc.tile_pool(name="sb", bufs=1) as sb,
            tc.tile_pool(name="work", bufs=6) as work,
            tc.tile_pool(name="pacc", bufs=1, space="PSUM") as pacc,
            tc.tile_pool(name="pcyc", bufs=4, space="PSUM") as pcyc,
        ):
            ident_bf = const.tile([128, 128], BF16)  # for w_gex/w_comp transposes
            make_identity(nc, ident_bf[:])
            ones_col = const.tile([128, 1], F32)     # f32 lhsT for LN stat sums
            nc.vector.memset(ones_col[:], 1.0)
            ones_col8 = const.tile([128, 1], F8)     # fp8 rhs for ppi row sums
            nc.gpsimd.memset(ones_col8[:], 1.0)
            ones_bf = const.tile([128, 1], BF16)     # bf16 lhsT/rhs broadcasts
            nc.gpsimd.memset(ones_bf[:], 1.0)
            ones_row = const.tile([1, 128], F32)     # f32 lhsT partition-bcast
            nc.vector.memset(ones_row[:], 1.0)
            ones_row_bf = const.tile([1, CH], BF16)
            nc.gpsimd.memset(ones_row_bf[:], 1.0)
            eps_t = const.tile([1, 1], F32)
            nc.vector.memset(eps_t[:], LN_EPS)
            # ACT table preloads: Exp and Sqrt live in different act-func
            # sets (1283ns load each).  Pay for Exp's at t~0 and Sqrt's
            # mid-kernel on an idle ACT, instead of on the critical path.
            # (Relu/Copy are members of both sets.)
            atl = const.tile([1, 1], F32)
            nc.vector.memset(atl[:], 1.0)
            atl2 = const.tile([1, 1], F32)
            nc.scalar.activation(atl2[:], atl[:], AF.Exp)

            _cyc_n = [0]

            def cyc(shape, dtype=F32):
                _cyc_n[0] += 1
                return pcyc.tile(shape, dtype, tag="cyc", name=f"cyc{_cyc_n[0]}")

            # persistent PSUM
            u_ps = pacc.tile([H, BL], F32, tag="u")
            stats = pacc.tile([1, 2 * BL], F32, tag="st")   # [x | x2]
            prsc_ps = pacc.tile([128, NGT], F32, tag="prs")

            # output staging (memset: tail partitions of tile 7 stay unread
            # by the host but must be finite for the DMA)
            outs_sb = sb.tile([128, 128], F32)
            nc.vector.memset(outs_sb[:], 0.0)

            # ============ loads ============
            # Issue cost per DMA is ~1.3us of the issuing engine's SEQ (HWDGE)
            # or ~1us of Pool (SWDGE), while all transfers serialize on the
            # shared DMA engines.  Spread issues across queues so the stream
            # is transfer-bound, and order arrivals by consumer priority.
            p50 = sb.tile([NA, P50_W], BF16)
            nc.sync.dma_start(out=p50[:], in_=pack50[:, :])
            p64 = sb.tile([CH, G], BF16)
            nc.scalar.dma_start(out=p64[:], in_=pack64[:, :])
            p128 = sb.tile([128, P128_W], BF16)
            nc.gpsimd.dma_start(out=p128[:], in_=pack128[:, :])
            ppiT_sb = sb.tile([128, NGT, G], F8)
            nc.sync.dma_start(out=ppiT_sb[:82, 7, :], in_=ppiT8[896:G, :])
            nc.sync.dma_start(out=ppiT_sb[:, 0:4, :],
                              in_=ppiT8[0:512, :].rearrange("(t p) k -> p t k", p=128))
            nc.sync.dma_start(out=ppiT_sb[:, 4:7, :],
                              in_=ppiT8[512:896, :].rearrange("(t p) k -> p t k", p=128))
            pf = sb.tile([128, PF_W], F32)
            nc.gpsimd.dma_start(out=pf[:], in_=packf[:, :])
            p1 = sb.tile([1, P1_W], BF16)
            nc.gpsimd.dma_start(out=p1[:], in_=pack1[:, :])
            wff_sb = sb.tile([128, NGT, G], BF16)
            nc.sync.dma_start(out=wff_sb[:, 0:4, :],
                              in_=wffb[0:512, :].rearrange("(t p) k -> p t k", p=128))
            nc.sync.dma_start(out=wff_sb[:, 4:7, :],
                              in_=wffb[512:896, :].rearrange("(t p) k -> p t k", p=128))
            nc.sync.dma_start(out=wff_sb[:82, 7, :], in_=wffb[896:G, :])

            # views into the packs
            nfT = p50[:FEAT, P50_NF:P50_NF + BL * NA]
            adjT = p50[:, P50_ADJ:P50_ADJ + BL * NA]
            distT = p50[:, P50_DIST:P50_DIST + BL * NA]
            W1v = p50[:FEAT, P50_W1:P50_W1 + CH]
            maskT = p50[:, P50_MASK:P50_MASK + BL]
            doseT = p1[0:1, P1_DO:P1_DO + BL]
            timeT = p1[0:1, P1_TI:P1_TI + BL]

            def wgv(t):
                return p128[:, P128_WG + t * H:P128_WG + (t + 1) * H]

            def wcv(t):
                return p128[:, P128_WC + t * H:P128_WC + (t + 1) * H]

            def bgv(t):  # f32 b_gex^T tile [128, BL]
                return pf[:, PF_BGT + t * BL:PF_BGT + (t + 1) * BL]

            def bgbv(t):  # bf16 b_gex^T tile [128, BL]
                return p128[:, P128_BGT + t * BL:P128_BGT + (t + 1) * BL]

            # ================= CCE =================
            # All contractions ride the PE with per-sample matmuls; the only
            # elementwise work is exp/mul for the edge weights and the final
            # masked-mean normalization.
            wmsg = sb.tile([NA, BL * NA], BF16)
            nc.scalar.activation(wmsg[:], distT, AF.Exp, scale=-1.0)
            nc.vector.tensor_mul(wmsg[:], wmsg[:], adjT)

            # h_b = relu(nf_b @ W1): [NA, CH] per sample  (lhsT = nfT slice)
            h_ps = cyc([NA, BL, CH])
            for b in range(BL):
                nc.tensor.matmul(h_ps[:, b, :], nfT[:, b * NA:(b + 1) * NA], W1v,
                                 start=True, stop=True)
            h_sb = sb.tile([NA, BL, CH], BF16)
            nc.scalar.activation(h_sb[:].rearrange("n b d -> n (b d)"),
                                 h_ps[:].rearrange("n b d -> n (b d)"), AF.Relu)

            # g_b[m] = sum_n mask[b,n] w[b,n,m] as columns [NA, 1] per sample
            g_ps = cyc([NA, BL])
            for b in range(BL):
                nc.tensor.matmul(g_ps[:, b:b + 1], wmsg[:, b * NA:(b + 1) * NA],
                                 maskT[:, b:b + 1], start=True, stop=True)
            g_cols = sb.tile([NA, BL], BF16)
            nc.vector.tensor_copy(g_cols[:], g_ps[:])

            # pooled_b[d] = sum_m g_b[m] h_b[m, d] -> [CH, 1] per sample
            pooled_ps = cyc([CH, BL])
            for b in range(BL):
                nc.tensor.matmul(pooled_ps[:, b:b + 1], h_sb[:, b, :],
                                 g_cols[:, b:b + 1], start=True, stop=True)

            ms_ps = cyc([1, BL])
            nc.tensor.matmul(ms_ps[:], ones_bf[:NA, :], maskT, start=True, stop=True)
            ms_sb = sb.tile([1, BL], F32)
            nc.vector.tensor_scalar_max(ms_sb[:], ms_ps[:], 1.0)
            rms_bf = sb.tile([1, BL], BF16)
            with nc.allow_low_precision(reason="mask-count reciprocal, exact for ones mask"):
                nc.vector.reciprocal(rms_bf[:], ms_sb[:])
            rb_ps = cyc([CH, BL])
            nc.tensor.matmul(rb_ps[:], ones_row_bf[:1, :], rms_bf[:], start=True, stop=True)
            pooledT = sb.tile([CH, BL], BF16)
            nc.vector.tensor_mul(pooledT[:], pooled_ps[:], rb_ps[:])

            # comp^T per gene tile: bf16 (for u / ssum) + f32 into outs
            compT = sb.tile([128, NGT, BL], BF16)
            for half in range(2):
                cT_ps = cyc([128, 4, BL])
                for j in range(4):
                    gt = half * 4 + j
                    gs, gn = GTS[gt]
                    nc.tensor.matmul(cT_ps[:gn, j, :], p64[:, gs:gs + gn], pooledT[:],
                                     start=True, stop=False)
                    nc.tensor.matmul(cT_ps[:gn, j, :], p1[0:1, P1_WD + gs:P1_WD + gs + gn],
                                     doseT, start=False, stop=False)
                    nc.tensor.matmul(cT_ps[:gn, j, :], p1[0:1, P1_WT + gs:P1_WT + gs + gn],
                                     timeT, start=False, stop=True)
                if half == 0:
                    nc.vector.tensor_copy(compT[:, 0:4, :], cT_ps[:])
                else:
                    nc.vector.tensor_copy(compT[:, 4:7, :], cT_ps[:, 0:3, :])
                    nc.vector.tensor_copy(compT[:82, 7, :], cT_ps[:82, 3, :])

            # ================= u = w_gex^T b_gex + w_comp^T comp =============
            for gt, (gs, gn) in enumerate(GTS):
                nc.tensor.matmul(u_ps[:], wgv(gt)[:gn, :], bgbv(gt)[:gn, :],
                                 start=(gt == 0), stop=False)
            for gt, (gs, gn) in enumerate(GTS):
                nc.tensor.matmul(u_ps[:], wcv(gt)[:gn, :], compT[:gn, gt, :],
                                 start=False, stop=(gt == NGT - 1))
            u_sb = sb.tile([H, BL], BF16)
            nc.scalar.activation(u_sb[:], u_ps[:], AF.Copy, scale=inv_sqrt_h)
            # anchor on ms_sb (positive, produced after the last Exp use) so
            # the scheduler cannot hoist this Sqrt above it (a hoisted Sqrt
            # would evict Exp's act table)
            nc.scalar.activation(atl2[:], ms_sb[0:1, 0:1], AF.Sqrt)

            # comp -> output staging (f32 from bf16 compT; off the ladder)
            nc.scalar.copy(outs_sb[:, 64:120],
                           compT[:, 0:7, :].rearrange("p t b -> p (t b)"))
            nc.scalar.copy(outs_sb[:82, 120:128], compT[:82, 7, :])

            # ====== w_gex/w_comp transposes + A/C matmuls (all tiles) ======
            # PE is in-order: emit everything that doesn't need ppi first,
            # then the ppi row-sum matmuls, then (post-ladder) the LN stats.
            AC = []  # per-tile PSUM [128, 2, BL]: [:,0]=A, [:,1]=C
            for gt2 in range(0, NGT, 2):
                gn0 = GTS[gt2][1]
                gn1 = GTS[gt2 + 1][1]
                wgc_ps = cyc([128, 4, 128], BF16)
                nc.tensor.transpose(wgc_ps[:, 0, :gn0], wgv(gt2)[:gn0, :],
                                    ident_bf[:gn0, :gn0])
                nc.tensor.transpose(wgc_ps[:, 1, :gn0], wcv(gt2)[:gn0, :],
                                    ident_bf[:gn0, :gn0])
                nc.tensor.transpose(wgc_ps[:, 2, :gn1], wgv(gt2 + 1)[:gn1, :],
                                    ident_bf[:gn1, :gn1])
                nc.tensor.transpose(wgc_ps[:, 3, :gn1], wcv(gt2 + 1)[:gn1, :],
                                    ident_bf[:gn1, :gn1])
                wgcT = work.tile([H, 4, 128], BF16, tag="wgcT")
                cp = nc.scalar.copy if gt2 % 4 == 0 else nc.vector.tensor_copy
                if gn1 == 128:
                    cp(wgcT[:].rearrange("p s h -> p (s h)"),
                       wgc_ps[:].rearrange("p s h -> p (s h)"))
                else:
                    cp(wgcT[:, 0:2, :].rearrange("p s h -> p (s h)"),
                       wgc_ps[:, 0:2, :].rearrange("p s h -> p (s h)"))
                    cp(wgcT[:, 2:4, :gn1], wgc_ps[:, 2:4, :gn1])
                for j in range(2):
                    gt = gt2 + j
                    gn = GTS[gt][1]
                    ac = cyc([128, 2, BL])
                    nc.tensor.matmul(ac[:gn, 0, :], wgcT[:, 2 * j, :gn], u_sb[:],
                                     start=True, stop=True)
                    nc.tensor.matmul(ac[:gn, 1, :], wgcT[:, 2 * j + 1, :gn], u_sb[:],
                                     start=True, stop=True)
                    AC.append(ac)

            # ========== ppi row sums via PE (ppi staged transposed) ==========
            # prs[g] = sum_k ppiT[k, g]: lhsT = ppiT tile [k, g-chunk],
            # rhs = ones -> out [g-chunk, 1]; accumulate over the 8 k-tiles.
            for nt, (ns, nn) in enumerate(GTS):
                for kt, (ks, kn) in enumerate(GTS):
                    nc.tensor.matmul(prsc_ps[:nn, nt:nt + 1],
                                     ppiT_sb[:kn, kt, ns:ns + nn], ones_col8[:kn, :],
                                     start=(kt == 0), stop=(kt == NGT - 1))
            prs = sb.tile([128, NGT], F32)
            nc.scalar.copy(prs[:, 0:NGT - 1], prsc_ps[:, 0:NGT - 1])
            nc.scalar.copy(prs[:82, NGT - 1:NGT], prsc_ps[:82, NGT - 1:NGT])

            # ====== score-sum ladder -> pred (gene-major) ======
            # predsq[:, t, 0:BL] = pred^T tile, [:, t, BL:2BL] = pred^2
            predsq = sb.tile([128, NGT, 2 * BL], F32)
            for gt, (gs, gn) in enumerate(GTS):
                ac = AC[gt]
                m1 = work.tile([128, BL], F32, tag="m1")
                nc.vector.tensor_mul(m1[:gn, :], bgbv(gt)[:gn, :], ac[:gn, 0, :])
                m2 = work.tile([128, BL], F32, tag="m2")
                nc.vector.tensor_mul(m2[:gn, :], compT[:gn, gt, :], ac[:gn, 1, :])
                nc.gpsimd.tensor_add(m1[:gn, :], m1[:gn, :], m2[:gn, :])
                # pred = b_gex * (ssum + prs)
                nc.gpsimd.scalar_tensor_tensor(predsq[:gn, gt, 0:BL], m1[:gn, :],
                                               prs[:gn, gt:gt + 1], bgv(gt)[:gn, :],
                                               op0=mybir.AluOpType.add,
                                               op1=mybir.AluOpType.mult)
                nc.gpsimd.tensor_mul(predsq[:gn, gt, BL:2 * BL],
                                     predsq[:gn, gt, 0:BL], predsq[:gn, gt, 0:BL])

            # ================= LN stats + LayerNorm + ReLU =================
            for gt, (gs, gn) in enumerate(GTS):
                nc.tensor.matmul(stats[:], ones_col[:gn, :], predsq[:gn, gt, :],
                                 start=(gt == 0), stop=(gt == NGT - 1))
            # mur = [mu | rstd] (f32, one row)
            mur = sb.tile([1, 2 * BL], F32)
            nc.vector.tensor_scalar_mul(mur[:], stats[:], 1.0 / G)
            mu2 = sb.tile([1, BL], F32)
            nc.vector.tensor_mul(mu2[:], mur[:, 0:BL], mur[:, 0:BL])
            nc.vector.tensor_sub(mur[:, BL:2 * BL], mur[:, BL:2 * BL], mu2[:])
            nc.scalar.activation(mur[:, BL:2 * BL], mur[:, BL:2 * BL], AF.Sqrt,
                                 bias=eps_t[:1, 0:1])
            nc.vector.reciprocal(mur[:, BL:2 * BL], mur[:, BL:2 * BL])
            mr_ps = cyc([128, 2 * BL])
            nc.tensor.matmul(mr_ps[:], ones_row[:], mur[:], start=True, stop=True)
            mr_sb = sb.tile([128, 2 * BL], F32)
            nc.scalar.copy(mr_sb[:], mr_ps[:])

            xn = sb.tile([128, NGT, BL], BF16)
            for gt, (gs, gn) in enumerate(GTS):
                eng = nc.vector if gt % 2 == 0 else nc.gpsimd
                mr = mr_ps if gt % 2 == 0 else mr_sb
                xm = work.tile([128, BL], F32, tag="xm")
                eng.tensor_sub(xm[:gn, :], predsq[:gn, gt, 0:BL], mr[:gn, 0:BL])
                eng.tensor_mul(xm[:gn, :], xm[:gn, :], mr[:gn, BL:2 * BL])
                eng.tensor_scalar(xm[:gn, :], xm[:gn, :],
                                  pf[:gn, PF_GAM + gt:PF_GAM + gt + 1],
                                  pf[:gn, PF_BET + gt:PF_BET + gt + 1],
                                  op0=mybir.AluOpType.mult,
                                  op1=mybir.AluOpType.add)
                eng.tensor_scalar_max(xn[:gn, gt, :], xm[:gn, :], 0.0)

            # ============ FFN, transposed: out^T[n,b] = sum_k Wff[k,n] x^T[k,b]
            # 7 concurrent single-bank PSUM groups (4 pcyc slots + the retired
            # u/prs/stats banks via tag reuse) accumulate kt-outer so the
            # matmuls chase the W_ff chunk stream; the 82-row tail tile runs
            # as a short second wave.
            fps = [cyc([128, BL]) for _ in range(4)]
            fps.append(pacc.tile([128, BL], F32, tag="u", name="ffn4"))
            fps.append(pacc.tile([128, BL], F32, tag="prs", name="ffn5"))
            fps.append(pacc.tile([128, BL], F32, tag="st", name="ffn6"))
            fps.append(cyc([128, BL]))
            for kt, (ks, kn) in enumerate(GTS):
                for nt, (ns, nn) in enumerate(GTS):
                    nc.tensor.matmul(fps[nt][:nn, :],
                                     wff_sb[:kn, kt, ns:ns + nn], xn[:kn, kt, :],
                                     start=(kt == 0), stop=(kt == NGT - 1))
            for nt, (ns, nn) in enumerate(GTS):
                eng = nc.scalar.copy if nt % 2 == 0 else nc.vector.tensor_copy
                eng(outs_sb[:nn, nt * BL:nt * BL + BL], fps[nt][:nn, :])
            nc.sync.dma_start(out=outs[:, :], in_=outs_sb[:])

    _split_excess_waits(nc)
    return nc


def _tile_gene_rows(a):
    """[G, X] -> [128, NGT, X] with zero padding (gene g = t*128 + p)."""
    x = a.shape[1]
    out = np.zeros((NGT * 128, x), a.dtype)
    out[:G] = a
    return np.ascontiguousarray(out.reshape(NGT, 128, x).transpose(1, 0, 2))


def make_in_maps(inputs):
    inputs = {k: np.asarray(v, dtype=np.float32) for k, v in inputs.items()}

    wg_t = _tile_gene_rows(inputs["w_gex"].astype(NP_BF16))      # [128,NGT,H]
    wc_t = _tile_gene_rows(inputs["w_comp"].astype(NP_BF16))
    pack128_w = np.concatenate(
        [wg_t.reshape(128, NGT * H), wc_t.reshape(128, NGT * H)], axis=1)
    gam_t = _tile_gene_rows(inputs["ln_gamma"].astype(np.float32)[:, None])
    bet_t = _tile_gene_rows(inputs["ln_beta"].astype(np.float32)[:, None])
    pack64 = np.ascontiguousarray(inputs["W2"].astype(NP_BF16))
    ppiT8 = np.ascontiguousarray(inputs["ppi_adj"].T).astype(NP_F8)
    wffb = inputs["W_ff"].astype(NP_BF16)

    in_maps = []
    for c in range(NCORES):
        s = slice(c * BL, (c + 1) * BL)
        p50 = np.zeros((NA, P50_W), NP_BF16)
        p50[:FEAT, P50_NF:P50_NF + BL * NA] = \
            inputs["node_feat"][s].transpose(2, 0, 1).reshape(FEAT, BL * NA)
        p50[:, P50_ADJ:P50_ADJ + BL * NA] = \
            inputs["adj_matrix"][s].transpose(1, 0, 2).reshape(NA, BL * NA)
        p50[:, P50_DIST:P50_DIST + BL * NA] = \
            inputs["dist_matrix"][s].transpose(1, 0, 2).reshape(NA, BL * NA)
        p50[:FEAT, P50_W1:P50_W1 + CH] = inputs["W1"]
        p50[:, P50_MASK:P50_MASK + BL] = inputs["mask"][s].T

        p1 = np.zeros((1, P1_W), NP_BF16)
        p1[0, P1_WD:P1_WD + G] = inputs["w_dose"][0]
        p1[0, P1_WT:P1_WT + G] = inputs["w_time"][0]
        p1[0, P1_DO:P1_DO + BL] = inputs["dose"][s, 0]
        p1[0, P1_TI:P1_TI + BL] = inputs["time"][s, 0]

        bgT = _tile_gene_rows(np.ascontiguousarray(inputs["b_gex"][s].T))
        pack128 = np.ascontiguousarray(np.concatenate(
            [pack128_w, bgT.astype(NP_BF16).reshape(128, NGT * BL)], axis=1))
        packf = np.ascontiguousarray(np.concatenate(
            [bgT.reshape(128, NGT * BL), gam_t.reshape(128, NGT),
             bet_t.reshape(128, NGT)], axis=1))
        in_maps.append({
            "pack50": p50,
            "pack1": p1,
            "pack64": pack64,
            "pack128": pack128,
            "packf": packf,
            "ppiT8": ppiT8,
            "wffb": wffb,
        })
    return in_maps


def _unpack_outs(arr):
    """[128, 128] f32 -> (pred [BL, G], comp [BL, G])."""
    def gm(cols):
        a = cols.reshape(128, NGT, BL)
        full = np.concatenate(
            [a[:, :7, :].transpose(1, 0, 2).reshape(7 * 128, BL), a[:82, 7, :]], 0)
        return np.ascontiguousarray(full.T)
    return gm(arr[:, 0:64]), gm(arr[:, 64:128])


def kernel(**inputs):
    nc = build_nc()
    in_maps = make_in_maps(inputs)
    r = run_bass_kernel_spmd(nc, in_maps, list(range(NCORES)))
    preds, comps = zip(*(_unpack_outs(r.results[c]["outs"]) for c in range(NCORES)))
    return np.concatenate(preds, 0), np.concatenate(comps, 0)


# revision 22
# speedup vs baseline: 1.0263x; 1.0263x over previous
"""Trainium2 Bass kernel for nn_CSG2A_net (gnn_message_passing).

Math (algebraically identical to the reference; the [B,G,G] score tensor is
never materialized):
  CCE:  h = relu(node_feat @ W1); w = adj*exp(-dist)
        g[b,m] = sum_n mask[b,n] * w[b,n,m]
        pooled[d,b] = (sum_m g[b,m] h[b,m,d]) / clip(sum_n mask[b,n], 1)
        comp = pooled @ W2 + dose @ w_dose + time @ w_time
  score.sum(-1)[b,g] = q[b,g,:] . u[b,:] / sqrt(H),  u = b_gex@w_gex + comp@w_comp
  pred = b_gex * (ssum + ppi_adj.sum(-1));  out = relu(LN(pred)) @ W_ff

Sharding: data-parallel over batch across 8 cores (8 samples each), weights
replicated.  On-chip layout is gene-major ([G-tile partitions x batch free]).

The cost structure on TRN2 is dominated by serialized HBM DMA (~360 B/ns all
queues combined), so the kernel minimizes DMA bytes and DMA count:
  - weights are down-cast host-side: w_gex/w_comp/W_ff/CCE weights to bf16,
    ppi_adj to fp8(e3m4) (it only feeds row-sums; quantization error on a
    978-element sum of U[0,1) values is ~0.04% of the sum)
  - ppi is staged TRANSPOSED so its row sums contract over the partition dim:
    64 rank-reduced PE matmuls against a ones vector instead of ~8us of
    DVE/ACT free-dim reductions
  - all small inputs ride in 4 packed images of the SBUF destination tiles
    (one DMA each); outputs pack into one [128,128] f32 tile (one DMA)
  - FFN runs transposed (out^T = W_ff^T x^T per gene tile) so each matmul
    moves only 8 rows; W_ff streams in 3 chunks overlapped with compute
"""

import numpy as np
import ml_dtypes

import concourse.bass as bass
import concourse.mybir as mybir
import concourse.tile as tile
from concourse.bass_utils import run_bass_kernel_spmd
from concourse.masks import make_identity

F32 = mybir.dt.float32
BF16 = mybir.dt.bfloat16
F8 = mybir.dt.float8e3
AF = mybir.ActivationFunctionType

NP_BF16 = ml_dtypes.bfloat16
NP_F8 = ml_dtypes.float8_e3m4

G, H, NA, FEAT, CH = 978, 128, 50, 34, 64
B, NCORES = 64, 8
BL = B // NCORES  # per-core batch
LN_EPS = 1e-5
# gene-dim tiles: 7 x 128 + 82
GTS = [(i * 128, 128) for i in range(7)] + [(896, 82)]
NGT = len(GTS)

# pack50 column layout: nfT | adjT | distT | W1 | maskT
P50_NF, P50_ADJ, P50_DIST, P50_W1, P50_MASK = 0, 400, 800, 1200, 1264
P50_W = 1272
# pack1 column layout: w_dose | w_time | doseT | timeT
P1_WD, P1_WT, P1_DO, P1_TI = 0, G, 2 * G, 2 * G + BL
P1_W = 2 * G + 2 * BL
# pack128 column layout: w_gex tiles | w_comp tiles | b_gex^T (bf16)
P128_WG, P128_WC, P128_BGT = 0, NGT * H, 2 * NGT * H
P128_W = 2 * NGT * H + NGT * BL
# packf (f32) column layout: b_gex^T tiles | ln_gamma | ln_beta
PF_BGT, PF_GAM, PF_BET = 0, NGT * BL, NGT * BL + NGT
PF_W = NGT * BL + 2 * NGT

_DMA_ZERO_WAIT = ("InstDMACopy", "InstDMATransposeAnt", "InstTriggeredCopy")


def _split_excess_waits(nc):
    """walrus in this container accepts at most 1 inline sync-wait per
    instruction (0 for DMA).  Move excess waits onto same-engine nops
    inserted immediately before the overloaded instruction."""

    def make_nop(engine):
        bi = nc.engines[engine].nop(nofuse=True)
        ins = bi.ins
        lst = nc.cur_bb.bb.instructions
        assert lst[-1] is ins
        lst.pop()
        return ins

    for bb in nc.main_func.blocks:
        lst = bb.instructions
        i = 0
        while i < len(lst):
            ins = lst[i]
            si = getattr(ins, "sync_info", None)
            waits = list(si.on_wait) if (si and si.on_wait) else []
            limit = 0 if type(ins).__name__ in _DMA_ZERO_WAIT else 1
            if len(waits) > limit:
                keep = waits[len(waits) - limit:] if limit else []
                excess = waits[: len(waits) - limit]
                si.on_wait = keep
                pos = i
                for w in excess:
                    nop = make_nop(ins.engine)
                    nop.sync_info = mybir.SyncInfo(on_wait=[w], on_update=[])
                    lst.insert(pos, nop)
                    pos += 1
                    i += 1
            i += 1


def build_nc():
    nc = bass.Bass()

    # ---- kernel I/O (per-core; all host-packed) ----
    pack50 = nc.dram_tensor("pack50", [NA, P50_W], BF16, kind="ExternalInput")
    pack1 = nc.dram_tensor("pack1", [1, P1_W], BF16, kind="ExternalInput")
    pack64 = nc.dram_tensor("pack64", [CH, G], BF16, kind="ExternalInput")
    pack128 = nc.dram_tensor("pack128", [128, P128_W], BF16, kind="ExternalInput")
    packf = nc.dram_tensor("packf", [128, PF_W], F32, kind="ExternalInput")
    ppiT8 = nc.dram_tensor("ppiT8", [G, G], F8, kind="ExternalInput")
    wffb = nc.dram_tensor("wffb", [G, G], BF16, kind="ExternalInput")
    # outs: cols [0:64] = pred^T tiles (t*BL+b), [64:128] = comp^T tiles
    outs = nc.dram_tensor("outs", [128, 128], F32, kind="ExternalOutput")

    inv_sqrt_h = 1.0 / float(np.sqrt(H))

    with tile.TileContext(nc) as tc:
        with (
            tc.tile_pool(name="const", bufs=1) as const,
            tc.tile_pool(name="sb", bufs=1) as sb,
            tc.tile_pool(name="work", bufs=6) as work,
            tc.tile_pool(name="pacc", bufs=1, space="PSUM") as pacc,
            tc.tile_pool(name="pcyc", bufs=4, space="PSUM") as pcyc,
        ):
            ident_bf = const.tile([128, 128], BF16)  # for w_gex/w_comp transposes
            make_identity(nc, ident_bf[:])
            ones_col = const.tile([128, 1], F32)     # f32 lhsT for LN stat sums
            nc.vector.memset(ones_col[:], 1.0)
            ones_col8 = const.tile([128, 1], F8)     # fp8 rhs for ppi row sums
            nc.gpsimd.memset(ones_col8[:], 1.0)
            ones_bf = const.tile([128, 1], BF16)     # bf16 lhsT/rhs broadcasts
            nc.gpsimd.memset(ones_bf[:], 1.0)
            ones_row = const.tile([1, 128], F32)     # f32 lhsT partition-bcast
            nc.vector.memset(ones_row[:], 1.0)
            ones_row_bf = const.tile([1, CH], BF16)
            nc.gpsimd.memset(ones_row_bf[:], 1.0)
            eps_t = const.tile([1, 1], F32)
            nc.vector.memset(eps_t[:], LN_EPS)
            # ACT table preloads: Exp and Sqrt live in different act-func
            # sets (1283ns load each).  Pay for Exp's at t~0 and Sqrt's
            # mid-kernel on an idle ACT, instead of on the critical path.
            # (Relu/Copy are members of both sets.)
            atl = const.tile([1, 1], F32)
            nc.vector.memset(atl[:], 1.0)
            atl2 = const.tile([1, 1], F32)
            nc.scalar.activation(atl2[:], atl[:], AF.Exp)

            _cyc_n = [0]

            def cyc(shape, dtype=F32):
                _cyc_n[0] += 1
                return pcyc.tile(shape, dtype, tag="cyc", name=f"cyc{_cyc_n[0]}")

            # persistent PSUM
            u_ps = pacc.tile([H, BL], F32, tag="u")
            stats = pacc.tile([1, 2 * BL], F32, tag="st")   # [x | x2]
            prsc_ps = pacc.tile([128, NGT], F32, tag="prs")

            # output staging (memset: tail partitions of tile 7 stay unread
            # by the host but must be finite for the DMA)
            outs_sb = sb.tile([128, 128], F32)
            nc.vector.memset(outs_sb[:], 0.0)

            # ============ loads ============
            # Issue cost per DMA is ~1.3us of the issuing engine's SEQ (HWDGE)
            # or ~1us of Pool (SWDGE), while all transfers serialize on the
            # shared DMA engines.  Spread issues across queues so the stream
            # is transfer-bound, and order arrivals by consumer priority.
            p50 = sb.tile([NA, P50_W], BF16)
            nc.sync.dma_start(out=p50[:], in_=pack50[:, :])
            p64 = sb.tile([CH, G], BF16)
            nc.scalar.dma_start(out=p64[:], in_=pack64[:, :])
            p128 = sb.tile([128, P128_W], BF16)
            nc.gpsimd.dma_start(out=p128[:], in_=pack128[:, :])
            ppiT_sb = sb.tile([128, NGT, G], F8)
            nc.sync.dma_start(out=ppiT_sb[:82, 7, :], in_=ppiT8[896:G, :])
            nc.sync.dma_start(out=ppiT_sb[:, 0:4, :],
                              in_=ppiT8[0:512, :].rearrange("(t p) k -> p t k", p=128))
            nc.sync.dma_start(out=ppiT_sb[:, 4:7, :],
                              in_=ppiT8[512:896, :].rearrange("(t p) k -> p t k", p=128))
            pf = sb.tile([128, PF_W], F32)
            nc.gpsimd.dma_start(out=pf[:], in_=packf[:, :])
            p1 = sb.tile([1, P1_W], BF16)
            nc.gpsimd.dma_start(out=p1[:], in_=pack1[:, :])
            wff_sb = sb.tile([128, NGT, G], BF16)
            nc.sync.dma_start(out=wff_sb[:, 0:4, :],
                              in_=wffb[0:512, :].rearrange("(t p) k -> p t k", p=128))
            nc.sync.dma_start(out=wff_sb[:, 4:7, :],
                              in_=wffb[512:896, :].rearrange("(t p) k -> p t k", p=128))
            nc.sync.dma_start(out=wff_sb[:82, 7, :], in_=wffb[896:G, :])

            # views into the packs
            nfT = p50[:FEAT, P50_NF:P50_NF + BL * NA]
            adjT = p50[:, P50_ADJ:P50_ADJ + BL * NA]
            distT = p50[:, P50_DIST:P50_DIST + BL * NA]
            W1v = p50[:FEAT, P50_W1:P50_W1 + CH]
            maskT = p50[:, P50_MASK:P50_MASK + BL]
            doseT = p1[0:1, P1_DO:P1_DO + BL]
            timeT = p1[0:1, P1_TI:P1_TI + BL]

            def wgv(t):
                return p128[:, P128_WG + t * H:P128_WG + (t + 1) * H]

            def wcv(t):
                return p128[:, P128_WC + t * H:P128_WC + (t + 1) * H]

            def bgv(t):  # f32 b_gex^T tile [128, BL]
                return pf[:, PF_BGT + t * BL:PF_BGT + (t + 1) * BL]

            def bgbv(t):  # bf16 b_gex^T tile [128, BL]
                return p128[:, P128_BGT + t * BL:P128_BGT + (t + 1) * BL]

            # ================= CCE =================
            # All contractions ride the PE with per-sample matmuls; the only
            # elementwise work is exp/mul for the edge weights and the final
            # masked-mean normalization.
            wmsg = sb.tile([NA, BL * NA], BF16)
            nc.scalar.activation(wmsg[:], distT, AF.Exp, scale=-1.0)
            nc.vector.tensor_mul(wmsg[:], wmsg[:], adjT)

            # h_b = relu(nf_b @ W1): [NA, CH] per sample  (lhsT = nfT slice)
            h_ps = cyc([NA, BL, CH])
            for b in range(BL):
                nc.tensor.matmul(h_ps[:, b, :], nfT[:, b * NA:(b + 1) * NA], W1v,
                                 start=True, stop=True)
            h_sb = sb.tile([NA, BL, CH], BF16)
            nc.scalar.activation(h_sb[:].rearrange("n b d -> n (b d)"),
                                 h_ps[:].rearrange("n b d -> n (b d)"), AF.Relu)

            # g_b[m] = sum_n mask[b,n] w[b,n,m] as columns [NA, 1] per sample
            g_ps = cyc([NA, BL])
            for b in range(BL):
                nc.tensor.matmul(g_ps[:, b:b + 1], wmsg[:, b * NA:(b + 1) * NA],
                                 maskT[:, b:b + 1], start=True, stop=True)
            g_cols = sb.tile([NA, BL], BF16)
            nc.vector.tensor_copy(g_cols[:], g_ps[:])

            # pooled_b[d] = sum_m g_b[m] h_b[m, d] -> [CH, 1] per sample
            pooled_ps = cyc([CH, BL])
            for b in range(BL):
                nc.tensor.matmul(pooled_ps[:, b:b + 1], h_sb[:, b, :],
                                 g_cols[:, b:b + 1], start=True, stop=True)

            ms_ps = cyc([1, BL])
            nc.tensor.matmul(ms_ps[:], ones_bf[:NA, :], maskT, start=True, stop=True)
            ms_sb = sb.tile([1, BL], F32)
            nc.vector.tensor_scalar_max(ms_sb[:], ms_ps[:], 1.0)
            rms_bf = sb.tile([1, BL], BF16)
            with nc.allow_low_precision(reason="mask-count reciprocal, exact for ones mask"):
                nc.vector.reciprocal(rms_bf[:], ms_sb[:])
            rb_ps = cyc([CH, BL])
            nc.tensor.matmul(rb_ps[:], ones_row_bf[:1, :], rms_bf[:], start=True, stop=True)
            pooledT = sb.tile([CH, BL], BF16)
            nc.vector.tensor_mul(pooledT[:], pooled_ps[:], rb_ps[:])

            # comp^T per gene tile: bf16 (for u / ssum) + f32 into outs
            compT = sb.tile([128, NGT, BL], BF16)
            for half in range(2):
                cT_ps = cyc([128, 4, BL])
                for j in range(4):
                    gt = half * 4 + j
                    gs, gn = GTS[gt]
                    nc.tensor.matmul(cT_ps[:gn, j, :], p64[:, gs:gs + gn], pooledT[:],
                                     start=True, stop=False)
                    nc.tensor.matmul(cT_ps[:gn, j, :], p1[0:1, P1_WD + gs:P1_WD + gs + gn],
                                     doseT, start=False, stop=False)
                    nc.tensor.matmul(cT_ps[:gn, j, :], p1[0:1, P1_WT + gs:P1_WT + gs + gn],
                                     timeT, start=False, stop=True)
                if half == 0:
                    nc.vector.tensor_copy(compT[:, 0:4, :], cT_ps[:])
                else:
                    nc.vector.tensor_copy(compT[:, 4:7, :], cT_ps[:, 0:3, :])
                    nc.vector.tensor_copy(compT[:82, 7, :], cT_ps[:82, 3, :])

            # ================= u = w_gex^T b_gex + w_comp^T comp =============
            for gt, (gs, gn) in enumerate(GTS):
                nc.tensor.matmul(u_ps[:], wgv(gt)[:gn, :], bgbv(gt)[:gn, :],
                                 start=(gt == 0), stop=False)
            for gt, (gs, gn) in enumerate(GTS):
                nc.tensor.matmul(u_ps[:], wcv(gt)[:gn, :], compT[:gn, gt, :],
                                 start=False, stop=(gt == NGT - 1))
            u_sb = sb.tile([H, BL], BF16)
            nc.scalar.activation(u_sb[:], u_ps[:], AF.Copy, scale=inv_sqrt_h)
            # anchor on ms_sb (positive, produced after the last Exp use) so
            # the scheduler cannot hoist this Sqrt above it (a hoisted Sqrt
            # would evict Exp's act table)
            nc.scalar.activation(atl2[:], ms_sb[0:1, 0:1], AF.Sqrt)

            # comp -> output staging (f32 from bf16 compT; off the ladder)
            nc.scalar.copy(outs_sb[:, 64:120],
                           compT[:, 0:7, :].rearrange("p t b -> p (t b)"))
            nc.scalar.copy(outs_sb[:82, 120:128], compT[:82, 7, :])

            # ====== w_gex/w_comp transposes + A/C matmuls (all tiles) ======
            # PE is in-order: emit everything that doesn't need ppi first,
            # then the ppi row-sum matmuls, then (post-ladder) the LN stats.
            AC = []  # per-tile PSUM [128, 2, BL]: [:,0]=A, [:,1]=C
            for gt2 in range(0, NGT, 2):
                gn0 = GTS[gt2][1]
                gn1 = GTS[gt2 + 1][1]
                wgc_ps = cyc([128, 4, 128], BF16)
                nc.tensor.transpose(wgc_ps[:, 0, :gn0], wgv(gt2)[:gn0, :],
                                    ident_bf[:gn0, :gn0])
                nc.tensor.transpose(wgc_ps[:, 1, :gn0], wcv(gt2)[:gn0, :],
                                    ident_bf[:gn0, :gn0])
                nc.tensor.transpose(wgc_ps[:, 2, :gn1], wgv(gt2 + 1)[:gn1, :],
                                    ident_bf[:gn1, :gn1])
                nc.tensor.transpose(wgc_ps[:, 3, :gn1], wcv(gt2 + 1)[:gn1, :],
                                    ident_bf[:gn1, :gn1])
                wgcT = work.tile([H, 4, 128], BF16, tag="wgcT")
                cp = nc.scalar.copy if gt2 % 4 == 0 else nc.vector.tensor_copy
                if gn1 == 128:
                    cp(wgcT[:].rearrange("p s h -> p (s h)"),
                       wgc_ps[:].rearrange("p s h -> p (s h)"))
                else:
                    cp(wgcT[:, 0:2, :].rearrange("p s h -> p (s h)"),
                       wgc_ps[:, 0:2, :].rearrange("p s h -> p (s h)"))
                    cp(wgcT[:, 2:4, :gn1], wgc_ps[:, 2:4, :gn1])
                for j in range(2):
                    gt = gt2 + j
                    gn = GTS[gt][1]
                    ac = cyc([128, 2, BL])
                    nc.tensor.matmul(ac[:gn, 0, :], wgcT[:, 2 * j, :gn], u_sb[:],
                                     start=True, stop=True)
                    nc.tensor.matmul(ac[:gn, 1, :], wgcT[:, 2 * j + 1, :gn], u_sb[:],
                                     start=True, stop=True)
                    AC.append(ac)

            # ========== ppi row sums via PE (ppi staged transposed) ==========
            # prs[g] = sum_k ppiT[k, g]: lhsT = ppiT tile [k, g-chunk],
            # rhs = ones -> out [g-chunk, 1]; accumulate over the 8 k-tiles.
            for nt, (ns, nn) in enumerate(GTS):
                for kt, (ks, kn) in enumerate(GTS):
                    nc.tensor.matmul(prsc_ps[:nn, nt:nt + 1],
                                     ppiT_sb[:kn, kt, ns:ns + nn], ones_col8[:kn, :],
                                     start=(kt == 0), stop=(kt == NGT - 1))
            prs = sb.tile([128, NGT], F32)
            nc.scalar.copy(prs[:, 0:NGT - 1], prsc_ps[:, 0:NGT - 1])
            nc.scalar.copy(prs[:82, NGT - 1:NGT], prsc_ps[:82, NGT - 1:NGT])

            # ====== score-sum ladder -> pred (gene-major) ======
            # predsq[:, t, 0:BL] = pred^T tile, [:, t, BL:2BL] = pred^2
            predsq = sb.tile([128, NGT, 2 * BL], F32)
            for gt, (gs, gn) in enumerate(GTS):
                ac = AC[gt]
                m1 = work.tile([128, BL], F32, tag="m1")
                nc.vector.tensor_mul(m1[:gn, :], bgbv(gt)[:gn, :], ac[:gn, 0, :])
                m2 = work.tile([128, BL], F32, tag="m2")
                nc.vector.tensor_mul(m2[:gn, :], compT[:gn, gt, :], ac[:gn, 1, :])
                nc.gpsimd.tensor_add(m1[:gn, :], m1[:gn, :], m2[:gn, :])
                # pred = b_gex * (ssum + prs)
                nc.gpsimd.scalar_tensor_tensor(predsq[:gn, gt, 0:BL], m1[:gn, :],
                                               prs[:gn, gt:gt + 1], bgv(gt)[:gn, :],
                                               op0=mybir.AluOpType.add,
                                               op1=mybir.AluOpType.mult)
                nc.gpsimd.tensor_mul(predsq[:gn, gt, BL:2 * BL],
                                     predsq[:gn, gt, 0:BL], predsq[:gn, gt, 0:BL])

            # ================= LN stats + LayerNorm + ReLU =================
            for gt, (gs, gn) in enumerate(GTS):
                nc.tensor.matmul(stats[:], ones_col[:gn, :], predsq[:gn, gt, :],
                                 start=(gt == 0), stop=(gt == NGT - 1))
            # mur = [mu | rstd] (f32, one row)
            mur = sb.tile([1, 2 * BL], F32)
            nc.vector.tensor_scalar_mul(mur[:], stats[:], 1.0 / G)
            mu2 = sb.tile([1, BL], F32)
            nc.vector.tensor_mul(mu2[:], mur[:, 0:BL], mur[:, 0:BL])
            nc.vector.tensor_sub(mur[:, BL:2 * BL], mur[:, BL:2 * BL], mu2[:])
            nc.scalar.activation(mur[:, BL:2 * BL], mur[:, BL:2 * BL], AF.Sqrt,
                                 bias=eps_t[:1, 0:1])
            nc.vector.reciprocal(mur[:, BL:2 * BL], mur[:, BL:2 * BL])
            mr_ps = pacc.tile([128, 2 * BL], F32, tag="mr")
            nc.tensor.matmul(mr_ps[:], ones_row[:], mur[:], start=True, stop=True)
            mr_sb = sb.tile([128, 2 * BL], F32)
            nc.scalar.copy(mr_sb[:], mr_ps[:])

            xn = sb.tile([128, NGT, BL], BF16)
            for gt, (gs, gn) in enumerate(GTS):
                eng = nc.vector if gt % 2 == 0 else nc.gpsimd
                xm = work.tile([128, BL], F32, tag="xm")
                eng.tensor_sub(xm[:gn, :], predsq[:gn, gt, 0:BL], mr_sb[:gn, 0:BL])
                eng.tensor_mul(xm[:gn, :], xm[:gn, :], mr_sb[:gn, BL:2 * BL])
                eng.tensor_scalar(xm[:gn, :], xm[:gn, :],
                                  pf[:gn, PF_GAM + gt:PF_GAM + gt + 1],
                                  pf[:gn, PF_BET + gt:PF_BET + gt + 1],
                                  op0=mybir.AluOpType.mult,
                                  op1=mybir.AluOpType.add)
                eng.tensor_scalar_max(xn[:gn, gt, :], xm[:gn, :], 0.0)

            # ============ FFN, transposed: out^T[n,b] = sum_k Wff[k,n] x^T[k,b]
            # 7 concurrent single-bank PSUM groups (4 pcyc slots + the retired
            # u/prs/stats banks via tag reuse) accumulate kt-outer so the
            # matmuls chase the W_ff chunk stream; the 82-row tail tile runs
            # as a short second wave.
            fps = [cyc([128, BL]) for _ in range(4)]
            fps.append(pacc.tile([128, BL], F32, tag="u", name="ffn4"))
            fps.append(pacc.tile([128, BL], F32, tag="prs", name="ffn5"))
            fps.append(pacc.tile([128, BL], F32, tag="st", name="ffn6"))
            fps.append(pacc.tile([128, BL], F32, tag="mr", name="ffn7"))
            for kt, (ks, kn) in enumerate(GTS):
                for nt, (ns, nn) in enumerate(GTS):
                    nc.tensor.matmul(fps[nt][:nn, :],
                                     wff_sb[:kn, kt, ns:ns + nn], xn[:kn, kt, :],
                                     start=(kt == 0), stop=(kt == NGT - 1))
            for nt, (ns, nn) in enumerate(GTS):
                eng = nc.scalar.copy if nt % 2 == 0 else nc.vector.tensor_copy
                eng(outs_sb[:nn, nt * BL:nt * BL + BL], fps[nt][:nn, :])
            nc.sync.dma_start(out=outs[:, :], in_=outs_sb[:])

    _split_excess_waits(nc)
    return nc


def _tile_gene_rows(a):
    """[G, X] -> [128, NGT, X] with zero padding (gene g = t*128 + p)."""
    x = a.shape[1]
    out = np.zeros((NGT * 128, x), a.dtype)
    out[:G] = a
    return np.ascontiguousarray(out.reshape(NGT, 128, x).transpose(1, 0, 2))


def make_in_maps(inputs):
    inputs = {k: np.asarray(v, dtype=np.float32) for k, v in inputs.items()}

    wg_t = _tile_gene_rows(inputs["w_gex"].astype(NP_BF16))      # [128,NGT,H]
    wc_t = _tile_gene_rows(inputs["w_comp"].astype(NP_BF16))
    pack128_w = np.concatenate(
        [wg_t.reshape(128, NGT * H), wc_t.reshape(128, NGT * H)], axis=1)
    gam_t = _tile_gene_rows(inputs["ln_gamma"].astype(np.float32)[:, None])
    bet_t = _tile_gene_rows(inputs["ln_beta"].astype(np.float32)[:, None])
    pack64 = np.ascontiguousarray(inputs["W2"].astype(NP_BF16))
    ppiT8 = np.ascontiguousarray(inputs["ppi_adj"].T).astype(NP_F8)
    wffb = inputs["W_ff"].astype(NP_BF16)

    in_maps = []
    for c in range(NCORES):
        s = slice(c * BL, (c + 1) * BL)
        p50 = np.zeros((NA, P50_W), NP_BF16)
        p50[:FEAT, P50_NF:P50_NF + BL * NA] = \
            inputs["node_feat"][s].transpose(2, 0, 1).reshape(FEAT, BL * NA)
        p50[:, P50_ADJ:P50_ADJ + BL * NA] = \
            inputs["adj_matrix"][s].transpose(1, 0, 2).reshape(NA, BL * NA)
        p50[:, P50_DIST:P50_DIST + BL * NA] = \
            inputs["dist_matrix"][s].transpose(1, 0, 2).reshape(NA, BL * NA)
        p50[:FEAT, P50_W1:P50_W1 + CH] = inputs["W1"]
        p50[:, P50_MASK:P50_MASK + BL] = inputs["mask"][s].T

        p1 = np.zeros((1, P1_W), NP_BF16)
        p1[0, P1_WD:P1_WD + G] = inputs["w_dose"][0]
        p1[0, P1_WT:P1_WT + G] = inputs["w_time"][0]
        p1[0, P1_DO:P1_DO + BL] = inputs["dose"][s, 0]
        p1[0, P1_TI:P1_TI + BL] = inputs["time"][s, 0]

        bgT = _tile_gene_rows(np.ascontiguousarray(inputs["b_gex"][s].T))
        pack128 = np.ascontiguousarray(np.concatenate(
            [pack128_w, bgT.astype(NP_BF16).reshape(128, NGT * BL)], axis=1))
        packf = np.ascontiguousarray(np.concatenate(
            [bgT.reshape(128, NGT * BL), gam_t.reshape(128, NGT),
             bet_t.reshape(128, NGT)], axis=1))
        in_maps.append({
            "pack50": p50,
            "pack1": p1,
            "pack64": pack64,
            "pack128": pack128,
            "packf": packf,
            "ppiT8": ppiT8,
            "wffb": wffb,
        })
    return in_maps


def _unpack_outs(arr):
    """[128, 128] f32 -> (pred [BL, G], comp [BL, G])."""
    def gm(cols):
        a = cols.reshape(128, NGT, BL)
        full = np.concatenate(
            [a[:, :7, :].transpose(1, 0, 2).reshape(7 * 128, BL), a[:82, 7, :]], 0)
        return np.ascontiguousarray(full.T)
    return gm(arr[:, 0:64]), gm(arr[:, 64:128])


def kernel(**inputs):
    nc = build_nc()
    in_maps = make_in_maps(inputs)
    r = run_bass_kernel_spmd(nc, in_maps, list(range(NCORES)))
    preds, comps = zip(*(_unpack_outs(r.results[c]["outs"]) for c in range(NCORES)))
    return np.concatenate(preds, 0), np.concatenate(comps, 0)


# revision 23
# speedup vs baseline: 1.0740x; 1.0464x over previous
"""Trainium2 Bass kernel for nn_CSG2A_net (gnn_message_passing).

Math (algebraically identical to the reference; the [B,G,G] score tensor is
never materialized):
  CCE:  h = relu(node_feat @ W1); w = adj*exp(-dist)
        g[b,m] = sum_n mask[b,n] * w[b,n,m]
        pooled[d,b] = (sum_m g[b,m] h[b,m,d]) / clip(sum_n mask[b,n], 1)
        comp = pooled @ W2 + dose @ w_dose + time @ w_time
  score.sum(-1)[b,g] = q[b,g,:] . u[b,:] / sqrt(H),  u = b_gex@w_gex + comp@w_comp
  pred = b_gex * (ssum + ppi_adj.sum(-1));  out = relu(LN(pred)) @ W_ff

Sharding: data-parallel over batch across 8 cores (8 samples each), weights
replicated.  On-chip layout is gene-major ([G-tile partitions x batch free]).

The cost structure on TRN2 is dominated by serialized HBM DMA (~360 B/ns all
queues combined), so the kernel minimizes DMA bytes and DMA count:
  - weights are down-cast host-side: w_gex/w_comp/W_ff/CCE weights to bf16,
    ppi_adj to fp8(e3m4) (it only feeds row-sums; quantization error on a
    978-element sum of U[0,1) values is ~0.04% of the sum)
  - ppi is staged TRANSPOSED so its row sums contract over the partition dim:
    64 rank-reduced PE matmuls against a ones vector instead of ~8us of
    DVE/ACT free-dim reductions
  - all small inputs ride in 4 packed images of the SBUF destination tiles
    (one DMA each); outputs pack into one [128,128] f32 tile (one DMA)
  - FFN runs transposed (out^T = W_ff^T x^T per gene tile) so each matmul
    moves only 8 rows; W_ff streams in 3 chunks overlapped with compute
"""

import numpy as np
import ml_dtypes

import concourse.bass as bass
import concourse.mybir as mybir
import concourse.tile as tile
from concourse.bass_utils import run_bass_kernel_spmd
from concourse.masks import make_identity

F32 = mybir.dt.float32
BF16 = mybir.dt.bfloat16
F8 = mybir.dt.float8e3
AF = mybir.ActivationFunctionType

NP_BF16 = ml_dtypes.bfloat16
NP_F8 = ml_dtypes.float8_e3m4

G, H, NA, FEAT, CH = 978, 128, 50, 34, 64
B, NCORES = 64, 8
BL = B // NCORES  # per-core batch
LN_EPS = 1e-5
# gene-dim tiles: 7 x 128 + 82
GTS = [(i * 128, 128) for i in range(7)] + [(896, 82)]
NGT = len(GTS)

# pack50 column layout: nfT | adjT | distT | W1 | maskT
P50_NF, P50_ADJ, P50_DIST, P50_W1, P50_MASK = 0, 400, 800, 1200, 1264
P50_W = 1272
# pack1 column layout: w_dose | w_time | doseT | timeT
P1_WD, P1_WT, P1_DO, P1_TI = 0, G, 2 * G, 2 * G + BL
P1_W = 2 * G + 2 * BL
# pack128 column layout: w_gex tiles | w_comp tiles | b_gex^T (bf16)
P128_WG, P128_WC, P128_BGT = 0, NGT * H, 2 * NGT * H
P128_W = 2 * NGT * H + NGT * BL
# packf (f32) column layout: b_gex^T tiles | ln_gamma | ln_beta
PF_BGT, PF_GAM, PF_BET = 0, NGT * BL, NGT * BL + NGT
PF_W = NGT * BL + 2 * NGT

_DMA_ZERO_WAIT = ("InstDMACopy", "InstDMATransposeAnt", "InstTriggeredCopy")


def _split_excess_waits(nc):
    """walrus in this container accepts at most 1 inline sync-wait per
    instruction (0 for DMA).  Move excess waits onto same-engine nops
    inserted immediately before the overloaded instruction."""

    def make_nop(engine):
        bi = nc.engines[engine].nop(nofuse=True)
        ins = bi.ins
        lst = nc.cur_bb.bb.instructions
        assert lst[-1] is ins
        lst.pop()
        return ins

    for bb in nc.main_func.blocks:
        lst = bb.instructions
        i = 0
        while i < len(lst):
            ins = lst[i]
            si = getattr(ins, "sync_info", None)
            waits = list(si.on_wait) if (si and si.on_wait) else []
            limit = 0 if type(ins).__name__ in _DMA_ZERO_WAIT else 1
            if len(waits) > limit:
                keep = waits[len(waits) - limit:] if limit else []
                excess = waits[: len(waits) - limit]
                si.on_wait = keep
                pos = i
                for w in excess:
                    nop = make_nop(ins.engine)
                    nop.sync_info = mybir.SyncInfo(on_wait=[w], on_update=[])
                    lst.insert(pos, nop)
                    pos += 1
                    i += 1
            i += 1


def build_nc():
    nc = bass.Bass()

    # ---- kernel I/O (per-core; all host-packed) ----
    pack50 = nc.dram_tensor("pack50", [NA, P50_W], BF16, kind="ExternalInput")
    pack1 = nc.dram_tensor("pack1", [1, P1_W], BF16, kind="ExternalInput")
    pack64 = nc.dram_tensor("pack64", [CH, G], BF16, kind="ExternalInput")
    pack128 = nc.dram_tensor("pack128", [128, P128_W], BF16, kind="ExternalInput")
    packf = nc.dram_tensor("packf", [128, PF_W], F32, kind="ExternalInput")
    ppiT8 = nc.dram_tensor("ppiT8", [G, G], F8, kind="ExternalInput")
    wffb = nc.dram_tensor("wffb", [G, G], BF16, kind="ExternalInput")
    # outs: cols [0:64] = pred^T tiles (t*BL+b), [64:128] = comp^T tiles
    outs = nc.dram_tensor("outs", [128, 128], F32, kind="ExternalOutput")

    inv_sqrt_h = 1.0 / float(np.sqrt(H))

    with tile.TileContext(nc) as tc:
        with (
            tc.tile_pool(name="const", bufs=1) as const,
            tc.tile_pool(name="sb", bufs=1) as sb,
            tc.tile_pool(name="work", bufs=6) as work,
            tc.tile_pool(name="pacc", bufs=1, space="PSUM") as pacc,
            tc.tile_pool(name="pcyc", bufs=4, space="PSUM") as pcyc,
        ):
            ident_bf = const.tile([128, 128], BF16)  # for w_gex/w_comp transposes
            make_identity(nc, ident_bf[:])
            ones_col = const.tile([128, 1], F32)     # f32 lhsT for LN stat sums
            nc.vector.memset(ones_col[:], 1.0)
            ones_col8 = const.tile([128, 1], F8)     # fp8 rhs for ppi row sums
            nc.gpsimd.memset(ones_col8[:], 1.0)
            ones_bf = const.tile([128, 1], BF16)     # bf16 lhsT/rhs broadcasts
            nc.gpsimd.memset(ones_bf[:], 1.0)
            ones_row = const.tile([1, 128], F32)     # f32 lhsT partition-bcast
            nc.vector.memset(ones_row[:], 1.0)
            ones_row_bf = const.tile([1, CH], BF16)
            nc.gpsimd.memset(ones_row_bf[:], 1.0)
            eps_t = const.tile([1, 1], F32)
            nc.vector.memset(eps_t[:], LN_EPS)
            # ACT table preloads: Exp and Sqrt live in different act-func
            # sets (1283ns load each).  Pay for Exp's at t~0 and Sqrt's
            # mid-kernel on an idle ACT, instead of on the critical path.
            # (Relu/Copy are members of both sets.)
            atl = const.tile([1, 1], F32)
            nc.vector.memset(atl[:], 1.0)
            atl2 = const.tile([1, 1], F32)
            nc.scalar.activation(atl2[:], atl[:], AF.Exp)

            _cyc_n = [0]

            def cyc(shape, dtype=F32):
                _cyc_n[0] += 1
                return pcyc.tile(shape, dtype, tag="cyc", name=f"cyc{_cyc_n[0]}")

            # persistent PSUM
            u_ps = pacc.tile([H, BL], F32, tag="u")
            stats = pacc.tile([1, 2 * BL], F32, tag="st")   # [x | x2]
            prsc_ps = pacc.tile([128, NGT], F32, tag="prs")

            # output staging (memset: tail partitions of tile 7 stay unread
            # by the host but must be finite for the DMA)
            outs_sb = sb.tile([128, 128], F32)
            nc.vector.memset(outs_sb[:], 0.0)

            # ============ loads ============
            # Issue cost per DMA is ~1.3us of the issuing engine's SEQ (HWDGE)
            # or ~1us of Pool (SWDGE), while all transfers serialize on the
            # shared DMA engines.  Spread issues across queues so the stream
            # is transfer-bound, and order arrivals by consumer priority.
            p50 = sb.tile([NA, P50_W], BF16)
            nc.sync.dma_start(out=p50[:], in_=pack50[:, :])
            p64 = sb.tile([CH, G], BF16)
            nc.scalar.dma_start(out=p64[:], in_=pack64[:, :])
            p128 = sb.tile([128, P128_W], BF16)
            nc.gpsimd.dma_start(out=p128[:], in_=pack128[:, :])
            ppiT_sb = sb.tile([128, NGT, G], F8)
            nc.sync.dma_start(out=ppiT_sb[:82, 7, :], in_=ppiT8[896:G, :])
            nc.sync.dma_start(out=ppiT_sb[:, 0:4, :],
                              in_=ppiT8[0:512, :].rearrange("(t p) k -> p t k", p=128))
            nc.sync.dma_start(out=ppiT_sb[:, 4:7, :],
                              in_=ppiT8[512:896, :].rearrange("(t p) k -> p t k", p=128))
            pf = sb.tile([128, PF_W], F32)
            nc.gpsimd.dma_start(out=pf[:], in_=packf[:, :])
            p1 = sb.tile([1, P1_W], BF16)
            nc.gpsimd.dma_start(out=p1[:], in_=pack1[:, :])
            wff_sb = sb.tile([128, NGT, G], BF16)
            nc.sync.dma_start(out=wff_sb[:, 0:4, :],
                              in_=wffb[0:512, :].rearrange("(t p) k -> p t k", p=128))
            nc.sync.dma_start(out=wff_sb[:, 4:7, :],
                              in_=wffb[512:896, :].rearrange("(t p) k -> p t k", p=128))
            nc.sync.dma_start(out=wff_sb[:82, 7, :], in_=wffb[896:G, :])

            # views into the packs
            nfT = p50[:FEAT, P50_NF:P50_NF + BL * NA]
            adjT = p50[:, P50_ADJ:P50_ADJ + BL * NA]
            distT = p50[:, P50_DIST:P50_DIST + BL * NA]
            W1v = p50[:FEAT, P50_W1:P50_W1 + CH]
            maskT = p50[:, P50_MASK:P50_MASK + BL]
            doseT = p1[0:1, P1_DO:P1_DO + BL]
            timeT = p1[0:1, P1_TI:P1_TI + BL]

            def wgv(t):
                return p128[:, P128_WG + t * H:P128_WG + (t + 1) * H]

            def wcv(t):
                return p128[:, P128_WC + t * H:P128_WC + (t + 1) * H]

            def bgv(t):  # f32 b_gex^T tile [128, BL]
                return pf[:, PF_BGT + t * BL:PF_BGT + (t + 1) * BL]

            def bgbv(t):  # bf16 b_gex^T tile [128, BL]
                return p128[:, P128_BGT + t * BL:P128_BGT + (t + 1) * BL]

            # ================= CCE =================
            # All contractions ride the PE with per-sample matmuls; the only
            # elementwise work is exp/mul for the edge weights and the final
            # masked-mean normalization.
            wmsg = sb.tile([NA, BL * NA], BF16)
            nc.scalar.activation(wmsg[:], distT, AF.Exp, scale=-1.0)
            nc.vector.tensor_mul(wmsg[:], wmsg[:], adjT)

            # h_b = relu(nf_b @ W1): [NA, CH] per sample  (lhsT = nfT slice)
            h_ps = cyc([NA, BL, CH])
            for b in range(BL):
                nc.tensor.matmul(h_ps[:, b, :], nfT[:, b * NA:(b + 1) * NA], W1v,
                                 start=True, stop=True)
            h_sb = sb.tile([NA, BL, CH], BF16)
            nc.scalar.activation(h_sb[:].rearrange("n b d -> n (b d)"),
                                 h_ps[:].rearrange("n b d -> n (b d)"), AF.Relu)

            # g_b[m] = sum_n mask[b,n] w[b,n,m] as columns [NA, 1] per sample
            g_ps = cyc([NA, BL])
            for b in range(BL):
                nc.tensor.matmul(g_ps[:, b:b + 1], wmsg[:, b * NA:(b + 1) * NA],
                                 maskT[:, b:b + 1], start=True, stop=True)
            g_cols = sb.tile([NA, BL], BF16)
            nc.vector.tensor_copy(g_cols[:], g_ps[:])

            # pooled_b[d] = sum_m g_b[m] h_b[m, d] -> [CH, 1] per sample
            pooled_ps = cyc([CH, BL])
            for b in range(BL):
                nc.tensor.matmul(pooled_ps[:, b:b + 1], h_sb[:, b, :],
                                 g_cols[:, b:b + 1], start=True, stop=True)

            ms_ps = cyc([1, BL])
            nc.tensor.matmul(ms_ps[:], ones_bf[:NA, :], maskT, start=True, stop=True)
            ms_sb = sb.tile([1, BL], F32)
            nc.vector.tensor_scalar_max(ms_sb[:], ms_ps[:], 1.0)
            rms_bf = sb.tile([1, BL], BF16)
            with nc.allow_low_precision(reason="mask-count reciprocal, exact for ones mask"):
                nc.vector.reciprocal(rms_bf[:], ms_sb[:])
            rb_ps = cyc([CH, BL])
            nc.tensor.matmul(rb_ps[:], ones_row_bf[:1, :], rms_bf[:], start=True, stop=True)
            pooledT = sb.tile([CH, BL], BF16)
            nc.vector.tensor_mul(pooledT[:], pooled_ps[:], rb_ps[:])

            # comp^T per gene tile: bf16 (for u / ssum) + f32 into outs
            compT = sb.tile([128, NGT, BL], BF16)
            for half in range(2):
                cT_ps = cyc([128, 4, BL])
                for j in range(4):
                    gt = half * 4 + j
                    gs, gn = GTS[gt]
                    nc.tensor.matmul(cT_ps[:gn, j, :], p64[:, gs:gs + gn], pooledT[:],
                                     start=True, stop=False)
                    nc.tensor.matmul(cT_ps[:gn, j, :], p1[0:1, P1_WD + gs:P1_WD + gs + gn],
                                     doseT, start=False, stop=False)
                    nc.tensor.matmul(cT_ps[:gn, j, :], p1[0:1, P1_WT + gs:P1_WT + gs + gn],
                                     timeT, start=False, stop=True)
                if half == 0:
                    nc.vector.tensor_copy(compT[:, 0:4, :], cT_ps[:])
                else:
                    nc.vector.tensor_copy(compT[:, 4:7, :], cT_ps[:, 0:3, :])
                    nc.vector.tensor_copy(compT[:82, 7, :], cT_ps[:82, 3, :])

            # ================= u = w_gex^T b_gex + w_comp^T comp =============
            for gt, (gs, gn) in enumerate(GTS):
                nc.tensor.matmul(u_ps[:], wgv(gt)[:gn, :], bgbv(gt)[:gn, :],
                                 start=(gt == 0), stop=False)
            for gt, (gs, gn) in enumerate(GTS):
                nc.tensor.matmul(u_ps[:], wcv(gt)[:gn, :], compT[:gn, gt, :],
                                 start=False, stop=(gt == NGT - 1))
            u_sb = sb.tile([H, BL], BF16)
            nc.vector.tensor_scalar_mul(u_sb[:], u_ps[:], inv_sqrt_h)

            # comp -> output staging (f32 from bf16 compT; off the ladder)
            nc.scalar.copy(outs_sb[:, 64:120],
                           compT[:, 0:7, :].rearrange("p t b -> p (t b)"))
            nc.scalar.copy(outs_sb[:82, 120:128], compT[:82, 7, :])

            # ====== w_gex/w_comp transposes + A/C matmuls (all tiles) ======
            # PE is in-order: emit everything that doesn't need ppi first,
            # then the ppi row-sum matmuls, then (post-ladder) the LN stats.
            AC = []  # per-tile PSUM [128, 2, BL]: [:,0]=A, [:,1]=C
            for gt2 in range(0, NGT, 2):
                gn0 = GTS[gt2][1]
                gn1 = GTS[gt2 + 1][1]
                wgc_ps = cyc([128, 4, 128], BF16)
                nc.tensor.transpose(wgc_ps[:, 0, :gn0], wgv(gt2)[:gn0, :],
                                    ident_bf[:gn0, :gn0])
                nc.tensor.transpose(wgc_ps[:, 1, :gn0], wcv(gt2)[:gn0, :],
                                    ident_bf[:gn0, :gn0])
                nc.tensor.transpose(wgc_ps[:, 2, :gn1], wgv(gt2 + 1)[:gn1, :],
                                    ident_bf[:gn1, :gn1])
                nc.tensor.transpose(wgc_ps[:, 3, :gn1], wcv(gt2 + 1)[:gn1, :],
                                    ident_bf[:gn1, :gn1])
                wgcT = work.tile([H, 4, 128], BF16, tag="wgcT")
                cp = nc.scalar.copy if gt2 % 4 == 0 else nc.vector.tensor_copy
                if gn1 == 128:
                    cp(wgcT[:].rearrange("p s h -> p (s h)"),
                       wgc_ps[:].rearrange("p s h -> p (s h)"))
                else:
                    cp(wgcT[:, 0:2, :].rearrange("p s h -> p (s h)"),
                       wgc_ps[:, 0:2, :].rearrange("p s h -> p (s h)"))
                    cp(wgcT[:, 2:4, :gn1], wgc_ps[:, 2:4, :gn1])
                for j in range(2):
                    gt = gt2 + j
                    gn = GTS[gt][1]
                    ac = cyc([128, 2, BL])
                    nc.tensor.matmul(ac[:gn, 0, :], wgcT[:, 2 * j, :gn], u_sb[:],
                                     start=True, stop=True)
                    nc.tensor.matmul(ac[:gn, 1, :], wgcT[:, 2 * j + 1, :gn], u_sb[:],
                                     start=True, stop=True)
                    AC.append(ac)

            # ========== ppi row sums via PE (ppi staged transposed) ==========
            # prs[g] = sum_k ppiT[k, g]: lhsT = ppiT tile [k, g-chunk],
            # rhs = ones -> out [g-chunk, 1]; accumulate over the 8 k-tiles.
            for nt, (ns, nn) in enumerate(GTS):
                for kt, (ks, kn) in enumerate(GTS):
                    nc.tensor.matmul(prsc_ps[:nn, nt:nt + 1],
                                     ppiT_sb[:kn, kt, ns:ns + nn], ones_col8[:kn, :],
                                     start=(kt == 0), stop=(kt == NGT - 1))
            prs = sb.tile([128, NGT], F32)
            nc.scalar.copy(prs[:, 0:NGT - 1], prsc_ps[:, 0:NGT - 1])
            nc.scalar.copy(prs[:82, NGT - 1:NGT], prsc_ps[:82, NGT - 1:NGT])
            # Sqrt act-table preload, anchored on prs (positive; ready mid-
            # kernel in an idle ACT window, before the LN Sqrt needs it and
            # after the last Exp use)
            nc.scalar.activation(atl2[:], prs[0:1, 0:1], AF.Sqrt)

            # ====== score-sum ladder -> pred (gene-major) ======
            # predsq[:, t, 0:BL] = pred^T tile, [:, t, BL:2BL] = pred^2
            predsq = sb.tile([128, NGT, 2 * BL], F32)
            for gt, (gs, gn) in enumerate(GTS):
                ac = AC[gt]
                m1 = work.tile([128, BL], F32, tag="m1")
                nc.vector.tensor_mul(m1[:gn, :], bgbv(gt)[:gn, :], ac[:gn, 0, :])
                m2 = work.tile([128, BL], F32, tag="m2")
                nc.vector.tensor_mul(m2[:gn, :], compT[:gn, gt, :], ac[:gn, 1, :])
                nc.gpsimd.tensor_add(m1[:gn, :], m1[:gn, :], m2[:gn, :])
                # pred = b_gex * (ssum + prs)
                nc.gpsimd.scalar_tensor_tensor(predsq[:gn, gt, 0:BL], m1[:gn, :],
                                               prs[:gn, gt:gt + 1], bgv(gt)[:gn, :],
                                               op0=mybir.AluOpType.add,
                                               op1=mybir.AluOpType.mult)
                nc.gpsimd.tensor_mul(predsq[:gn, gt, BL:2 * BL],
                                     predsq[:gn, gt, 0:BL], predsq[:gn, gt, 0:BL])

            # ================= LN stats + LayerNorm + ReLU =================
            for gt, (gs, gn) in enumerate(GTS):
                nc.tensor.matmul(stats[:], ones_col[:gn, :], predsq[:gn, gt, :],
                                 start=(gt == 0), stop=(gt == NGT - 1))
            # mur = [mu | rstd] (f32, one row)
            mur = sb.tile([1, 2 * BL], F32)
            nc.vector.tensor_scalar_mul(mur[:], stats[:], 1.0 / G)
            mu2 = sb.tile([1, BL], F32)
            nc.vector.tensor_mul(mu2[:], mur[:, 0:BL], mur[:, 0:BL])
            nc.vector.tensor_sub(mur[:, BL:2 * BL], mur[:, BL:2 * BL], mu2[:])
            nc.scalar.activation(mur[:, BL:2 * BL], mur[:, BL:2 * BL], AF.Sqrt,
                                 bias=eps_t[:1, 0:1])
            nc.vector.reciprocal(mur[:, BL:2 * BL], mur[:, BL:2 * BL])
            mr_ps = pacc.tile([128, 2 * BL], F32, tag="mr")
            nc.tensor.matmul(mr_ps[:], ones_row[:], mur[:], start=True, stop=True)
            mr_sb = sb.tile([128, 2 * BL], F32)
            nc.scalar.copy(mr_sb[:], mr_ps[:])

            xn = sb.tile([128, NGT, BL], BF16)
            for gt, (gs, gn) in enumerate(GTS):
                eng = nc.vector if gt % 2 == 0 else nc.gpsimd
                xm = work.tile([128, BL], F32, tag="xm")
                eng.tensor_sub(xm[:gn, :], predsq[:gn, gt, 0:BL], mr_sb[:gn, 0:BL])
                eng.tensor_mul(xm[:gn, :], xm[:gn, :], mr_sb[:gn, BL:2 * BL])
                eng.tensor_scalar(xm[:gn, :], xm[:gn, :],
                                  pf[:gn, PF_GAM + gt:PF_GAM + gt + 1],
                                  pf[:gn, PF_BET + gt:PF_BET + gt + 1],
                                  op0=mybir.AluOpType.mult,
                                  op1=mybir.AluOpType.add)
                eng.tensor_scalar_max(xn[:gn, gt, :], xm[:gn, :], 0.0)

            # ============ FFN, transposed: out^T[n,b] = sum_k Wff[k,n] x^T[k,b]
            # 7 concurrent single-bank PSUM groups (4 pcyc slots + the retired
            # u/prs/stats banks via tag reuse) accumulate kt-outer so the
            # matmuls chase the W_ff chunk stream; the 82-row tail tile runs
            # as a short second wave.
            fps = [cyc([128, BL]) for _ in range(4)]
            fps.append(pacc.tile([128, BL], F32, tag="u", name="ffn4"))
            fps.append(pacc.tile([128, BL], F32, tag="prs", name="ffn5"))
            fps.append(pacc.tile([128, BL], F32, tag="st", name="ffn6"))
            fps.append(pacc.tile([128, BL], F32, tag="mr", name="ffn7"))
            for kt, (ks, kn) in enumerate(GTS):
                for nt, (ns, nn) in enumerate(GTS):
                    nc.tensor.matmul(fps[nt][:nn, :],
                                     wff_sb[:kn, kt, ns:ns + nn], xn[:kn, kt, :],
                                     start=(kt == 0), stop=(kt == NGT - 1))
            for nt, (ns, nn) in enumerate(GTS):
                eng = nc.scalar.copy if nt % 2 == 0 else nc.vector.tensor_copy
                eng(outs_sb[:nn, nt * BL:nt * BL + BL], fps[nt][:nn, :])
            nc.sync.dma_start(out=outs[:, :], in_=outs_sb[:])

    _split_excess_waits(nc)
    return nc


def _tile_gene_rows(a):
    """[G, X] -> [128, NGT, X] with zero padding (gene g = t*128 + p)."""
    x = a.shape[1]
    out = np.zeros((NGT * 128, x), a.dtype)
    out[:G] = a
    return np.ascontiguousarray(out.reshape(NGT, 128, x).transpose(1, 0, 2))


def make_in_maps(inputs):
    inputs = {k: np.asarray(v, dtype=np.float32) for k, v in inputs.items()}

    wg_t = _tile_gene_rows(inputs["w_gex"].astype(NP_BF16))      # [128,NGT,H]
    wc_t = _tile_gene_rows(inputs["w_comp"].astype(NP_BF16))
    pack128_w = np.concatenate(
        [wg_t.reshape(128, NGT * H), wc_t.reshape(128, NGT * H)], axis=1)
    gam_t = _tile_gene_rows(inputs["ln_gamma"].astype(np.float32)[:, None])
    bet_t = _tile_gene_rows(inputs["ln_beta"].astype(np.float32)[:, None])
    pack64 = np.ascontiguousarray(inputs["W2"].astype(NP_BF16))
    ppiT8 = np.ascontiguousarray(inputs["ppi_adj"].T).astype(NP_F8)
    wffb = inputs["W_ff"].astype(NP_BF16)

    in_maps = []
    for c in range(NCORES):
        s = slice(c * BL, (c + 1) * BL)
        p50 = np.zeros((NA, P50_W), NP_BF16)
        p50[:FEAT, P50_NF:P50_NF + BL * NA] = \
            inputs["node_feat"][s].transpose(2, 0, 1).reshape(FEAT, BL * NA)
        p50[:, P50_ADJ:P50_ADJ + BL * NA] = \
            inputs["adj_matrix"][s].transpose(1, 0, 2).reshape(NA, BL * NA)
        p50[:, P50_DIST:P50_DIST + BL * NA] = \
            inputs["dist_matrix"][s].transpose(1, 0, 2).reshape(NA, BL * NA)
        p50[:FEAT, P50_W1:P50_W1 + CH] = inputs["W1"]
        p50[:, P50_MASK:P50_MASK + BL] = inputs["mask"][s].T

        p1 = np.zeros((1, P1_W), NP_BF16)
        p1[0, P1_WD:P1_WD + G] = inputs["w_dose"][0]
        p1[0, P1_WT:P1_WT + G] = inputs["w_time"][0]
        p1[0, P1_DO:P1_DO + BL] = inputs["dose"][s, 0]
        p1[0, P1_TI:P1_TI + BL] = inputs["time"][s, 0]

        bgT = _tile_gene_rows(np.ascontiguousarray(inputs["b_gex"][s].T))
        pack128 = np.ascontiguousarray(np.concatenate(
            [pack128_w, bgT.astype(NP_BF16).reshape(128, NGT * BL)], axis=1))
        packf = np.ascontiguousarray(np.concatenate(
            [bgT.reshape(128, NGT * BL), gam_t.reshape(128, NGT),
             bet_t.reshape(128, NGT)], axis=1))
        in_maps.append({
            "pack50": p50,
            "pack1": p1,
            "pack64": pack64,
            "pack128": pack128,
            "packf": packf,
            "ppiT8": ppiT8,
            "wffb": wffb,
        })
    return in_maps


def _unpack_outs(arr):
    """[128, 128] f32 -> (pred [BL, G], comp [BL, G])."""
    def gm(cols):
        a = cols.reshape(128, NGT, BL)
        full = np.concatenate(
            [a[:, :7, :].transpose(1, 0, 2).reshape(7 * 128, BL), a[:82, 7, :]], 0)
        return np.ascontiguousarray(full.T)
    return gm(arr[:, 0:64]), gm(arr[:, 64:128])


def kernel(**inputs):
    nc = build_nc()
    in_maps = make_in_maps(inputs)
    r = run_bass_kernel_spmd(nc, in_maps, list(range(NCORES)))
    preds, comps = zip(*(_unpack_outs(r.results[c]["outs"]) for c in range(NCORES)))
    return np.concatenate(preds, 0), np.concatenate(comps, 0)
